# revision 1
# baseline (speedup 1.0000x reference)
"""TRN2 Bass kernel for nn_MultiPrecisionLinear (moe_routing).

Reference computation:
    xs = x.reshape(P, bpp, S, Din)            # P=8 paths
    W  = weight_bank[assigned_bits]           # [P, Dout, Din]
    out = einsum('pbsi,poi->pbso', xs, W) + bias

Sharding: path-parallel. Core p holds path p's batch slice
x_p = [bpp*S, Din] = [32768, 256] and its selected weight (transposed to
[Din, Dout] on host — a 256KB copy), plus a replicated bias.

Per-core dataflow (memory-bound target ~186us @ 358 GB/s):
  DMA 1MB chunks of x -> SBUF [128, CH, 256]
  per 128-row tile:  PE-transpose (fp32, via identity) -> PSUM
                     ACT copy PSUM -> SBUF rounding to fp32r
                     2 fp32r matmuls (K=256 split) -> PSUM [128 m, 256 o]
                     DVE add broadcast bias -> SBUF out chunk
  DMA 1MB chunks back to DRAM.

fp32r: full-rate PE (1 cyc/row vs fp32's 4) at ~1.5e-4 rel RMS error
(measured on HW; bf16 is 2.3e-3).
"""

import numpy as np

import concourse.bacc as bacc
import concourse.mybir as mybir
import concourse.tile as tile
from concourse.masks import make_identity

F32 = mybir.dt.float32
F32R = mybir.dt.float32r

# Problem geometry (hardcoded per spec).
P = 8          # paths == cores
BPP = 8        # batch per path
S = 4096
DIN = 256
DOUT = 256
M = BPP * S    # rows per core = 32768
NT = M // 128  # 128-row m-tiles per core = 256
CH = 8         # m-tiles per DMA chunk (1 MiB)

_CACHE = {}


def build_nc(nt=NT, ch=CH):
    """Build the per-core Bass/Tile program (identical on all 8 cores)."""
    key = (nt, ch)
    if key in _CACHE:
        return _CACHE[key]

    nc = bacc.Bacc("TRN2", target_bir_lowering=False, debug=False)
    x_d = nc.dram_tensor("x", [nt, 128, DIN], F32, kind="ExternalInput")
    w_d = nc.dram_tensor("w", [2, 128, DOUT], F32, kind="ExternalInput")
    bias_d = nc.dram_tensor("biasb", [128, DOUT], F32, kind="ExternalInput")
    out_d = nc.dram_tensor("out", [nt, 128, DOUT], F32, kind="ExternalOutput")

    nchunk = nt // ch
    assert nchunk * ch == nt

    with tile.TileContext(nc) as tc:
        with (
            tc.tile_pool(name="const", bufs=1) as const,
            tc.tile_pool(name="xin", bufs=3) as xin_pool,
            tc.tile_pool(name="oout", bufs=3) as oout_pool,
            tc.tile_pool(name="xt", bufs=4) as xt_pool,
            tc.tile_pool(name="pst", bufs=2, space="PSUM") as pst,
            tc.tile_pool(name="pso", bufs=3, space="PSUM") as pso,
        ):
            # One-time setup: weights (rounded to fp32r), bias, identity.
            w_sb = const.tile([128, 2, DOUT], F32, tag="w_sb")
            nc.sync.dma_start(w_sb[:], w_d[:].rearrange("c p n -> p c n"))
            w_r = const.tile([128, 2, DOUT], F32R, tag="w_r")
            nc.vector.tensor_copy(w_r[:], w_sb[:])
            bias_sb = const.tile([128, DOUT], F32, tag="bias_sb")
            nc.sync.dma_start(bias_sb[:], bias_d[:])
            ident = const.tile([128, 128], F32, tag="ident")
            make_identity(nc, ident[:])

            for c in range(nchunk):
                xin = xin_pool.tile([128, ch, DIN], F32, tag="xin")
                nc.sync.dma_start(
                    xin[:], x_d[c * ch : (c + 1) * ch].rearrange("n p i -> p n i")
                )
                oout = oout_pool.tile([128, ch, DOUT], F32, tag="oout")
                for j in range(ch):
                    x_sb = xin[:, j, :]
                    t0 = pst.tile([128, 128], F32, tag="t0")
                    t1 = pst.tile([128, 128], F32, tag="t1")
                    nc.tensor.transpose(t0[:], x_sb[:, 0:128], ident[:])
                    nc.tensor.transpose(t1[:], x_sb[:, 128:256], ident[:])
                    xt = xt_pool.tile([128, 2, 128], F32R, tag="xt")
                    nc.scalar.copy(xt[:, 0, :], t0[:])
                    nc.scalar.copy(xt[:, 1, :], t1[:])
                    po = pso.tile([128, DOUT], F32, tag="po")
                    nc.tensor.matmul(
                        po[:], xt[:, 0, :], w_r[:, 0, :], start=True, stop=False
                    )
                    nc.tensor.matmul(
                        po[:], xt[:, 1, :], w_r[:, 1, :], start=False, stop=True
                    )
                    nc.vector.tensor_add(oout[:, j, :], po[:], bias_sb[:])
                nc.sync.dma_start(
                    out_d[c * ch : (c + 1) * ch].rearrange("n p i -> p n i"), oout[:]
                )
    nc.compile()
    _CACHE[key] = nc
    return nc


def make_in_maps(x, weight_bank, bias, assigned_bits, nt=NT):
    """Host-side sharding: per-core input dicts."""
    x = np.asarray(x, dtype=np.float32)
    weight_bank = np.asarray(weight_bank, dtype=np.float32)
    bias = np.asarray(bias, dtype=np.float32)
    idx = np.asarray(assigned_bits).astype(np.int64)

    bias_b = np.ascontiguousarray(np.broadcast_to(bias[None, :], (128, DOUT)))
    xs = x.reshape(P, nt, 128, DIN)  # zero-copy view
    in_maps = []
    for p in range(P):
        w_io = np.ascontiguousarray(weight_bank[idx[p]].T)  # [Din, Dout]
        in_maps.append(
            {
                "x": xs[p],
                "w": w_io.reshape(2, 128, DOUT),
                "biasb": bias_b,
            }
        )
    return in_maps


def assemble_out(results, nt=NT):
    out = np.stack([r["out"].reshape(BPP, S, DOUT) for r in results])
    return np.ascontiguousarray(out.reshape(P * BPP, S, DOUT))


def kernel(x, weight_bank, bias, assigned_bits):
    from concourse.bass_utils import run_bass_kernel_spmd

    nc = build_nc()
    in_maps = make_in_maps(x, weight_bank, bias, assigned_bits)
    res = run_bass_kernel_spmd(nc, in_maps, core_ids=list(range(P)))
    return assemble_out(res.results)


# revision 2
# speedup vs baseline: 1.0337x; 1.0337x over previous
"""TRN2 Bass kernel for nn_MultiPrecisionLinear (moe_routing).

Reference computation:
    xs = x.reshape(P, bpp, S, Din)            # P=8 paths
    W  = weight_bank[assigned_bits]           # [P, Dout, Din]
    out = einsum('pbsi,poi->pbso', xs, W) + bias

Sharding: path-parallel. Core p holds path p's batch slice
[bpp*S, Din] = [32768, 256], its selected weight (as [Din, Dout]) and the
bias. All layout transposes happen on host, which makes the device kernel a
pure streaming matmul:

  per 512-column chunk m of xT (host-pretransposed, [2, 128, 32768] f32):
    DMA in  [128, 2, 512]            (2KB contiguous per partition)
    4 fp32r matmuls: out_T[oc] += W[ic, oc block].T-free @ xT[ic]  -> PSUM
    bias add fused with PSUM->SBUF move (ACT Identity for oc=0, DVE
    tensor_scalar_add for oc=1; bias is per-partition in this layout)
    DMA out [128, 2, 512] -> outT [2, 128, 32768]

fp32r: full-rate PE (1 cyc/row) at ~1.5e-4 rel RMS error (HW-measured;
fp32 is 4x slower, bf16 is 16x less accurate). DRAM inputs are declared
float32r with raw f32 bytes — HW rounds internally, verified equivalent
to explicit on-device rounding.
"""

import numpy as np

import concourse.bacc as bacc
import concourse.mybir as mybir
import concourse.tile as tile

F32 = mybir.dt.float32
F32R = mybir.dt.float32r
AF = mybir.ActivationFunctionType

# Problem geometry (hardcoded per spec).
P = 8          # paths == cores
BPP = 8        # batch per path
S = 4096
DIN = 256
DOUT = 256
M = BPP * S    # rows per core = 32768
MC = 512       # columns of xT per chunk

_CACHE = {}


def build_nc(m=M, mc=MC):
    key = (m, mc)
    if key in _CACHE:
        return _CACHE[key]

    nc = bacc.Bacc("TRN2", target_bir_lowering=False, debug=False)
    xt_d = nc.dram_tensor("xt", [2, 128, m], F32R, kind="ExternalInput")
    w_d = nc.dram_tensor("w", [2, 128, DOUT], F32R, kind="ExternalInput")
    bias_d = nc.dram_tensor("bias2", [2, 128], F32, kind="ExternalInput")
    out_d = nc.dram_tensor("outT", [2, 128, m], F32, kind="ExternalOutput")

    nchunk = m // mc
    assert nchunk * mc == m

    with tile.TileContext(nc) as tc:
        with (
            tc.tile_pool(name="const", bufs=1) as const,
            tc.tile_pool(name="xin", bufs=4) as xin_pool,
            tc.tile_pool(name="oout", bufs=4) as oout_pool,
            tc.tile_pool(name="psum", bufs=2, space="PSUM") as psum,
        ):
            w_sb = const.tile([128, 2, DOUT], F32R, tag="w_sb")
            nc.sync.dma_start(w_sb[:], w_d[:].rearrange("c p n -> p c n"))
            bias_sb = const.tile([128, 2], F32, tag="bias_sb")
            nc.sync.dma_start(bias_sb[:], bias_d[:].rearrange("c p -> p c"))

            for c in range(nchunk):
                sl = slice(c * mc, (c + 1) * mc)
                xt = xin_pool.tile([128, 2, mc], F32R, tag="xt")
                nc.sync.dma_start(xt[:], xt_d[:, :, sl].rearrange("c p m -> p c m"))
                osb = oout_pool.tile([128, 2, mc], F32, tag="osb")
                for oc in range(2):
                    po = psum.tile([128, mc], F32, tag=f"po{oc}")
                    for ic in range(2):
                        nc.tensor.matmul(
                            po[:],
                            w_sb[:, ic, oc * 128 : (oc + 1) * 128],
                            xt[:, ic, :],
                            start=(ic == 0),
                            stop=(ic == 1),
                        )
                    if oc == 0:
                        nc.scalar.activation(
                            osb[:, oc, :], po[:], AF.Identity,
                            bias=bias_sb[:, oc : oc + 1],
                        )
                    else:
                        nc.vector.tensor_scalar_add(
                            osb[:, oc, :], po[:], bias_sb[:, oc : oc + 1]
                        )
                nc.sync.dma_start(
                    out_d[:, :, sl].rearrange("c p m -> p c m"), osb[:]
                )
    nc.compile()
    _CACHE[key] = nc
    return nc


def make_in_maps(x, weight_bank, bias, assigned_bits, m=M):
    """Host-side sharding + layout: per-core input dicts."""
    x = np.asarray(x, dtype=np.float32)
    weight_bank = np.asarray(weight_bank, dtype=np.float32)
    bias = np.asarray(bias, dtype=np.float32)
    idx = np.asarray(assigned_bits).astype(np.int64)

    bias2 = np.ascontiguousarray(bias.reshape(2, 128))
    xs = x.reshape(P, m, DIN)
    in_maps = []
    for p in range(P):
        xt = np.ascontiguousarray(xs[p].T).reshape(2, 128, m)  # [ic, i, m]
        w_io = np.ascontiguousarray(weight_bank[idx[p]].T)     # [Din, Dout]
        in_maps.append(
            {
                "xt": xt,
                "w": w_io.reshape(2, 128, DOUT),
                "bias2": bias2,
            }
        )
    return in_maps


def assemble_out(results, m=M):
    outs = []
    for r in results:
        ot = r["outT"].reshape(DOUT, m)  # [o, m]
        outs.append(np.ascontiguousarray(ot.T))  # [m, o]
    out = np.stack(outs)  # [P, m, DOUT]
    return np.ascontiguousarray(out.reshape(P * BPP, S, DOUT))


def kernel(x, weight_bank, bias, assigned_bits):
    from concourse.bass_utils import run_bass_kernel_spmd

    nc = build_nc()
    in_maps = make_in_maps(x, weight_bank, bias, assigned_bits)
    res = run_bass_kernel_spmd(nc, in_maps, core_ids=list(range(P)))
    return assemble_out(res.results)


# revision 4
# speedup vs baseline: 1.2361x; 1.1958x over previous
"""TRN2 Bass kernel for nn_MultiPrecisionLinear (moe_routing).

Reference computation:
    xs = x.reshape(P, bpp, S, Din)            # P=8 paths
    W  = weight_bank[assigned_bits]           # [P, Dout, Din]
    out = einsum('pbsi,poi->pbso', xs, W) + bias

Sharding: path-parallel. Core p holds path p's batch slice
[bpp*S, Din] = [32768, 256], its selected weight (as [Din, Dout]) and the
bias. All layout work happens on host so the device kernel is a pure
streaming matmul over fp32r:

  x is pre-transposed AND pre-chunked on host into contiguous 1MB blocks
  xt[c] = [128(i%128), 2(i//128), MC(m)]  -> each DMA reads one contiguous
  block, 8KB contiguous per partition (minimal descriptor count).

  per chunk c:
    DMA in  xt[c] (1MB, Sync HWDGE)
    8 fp32r matmuls (2 oc x 2 ic x 2 halves, N=512) -> out_T in PSUM
    bias add fused with PSUM->SBUF move (ACT Identity for oc=0, DVE
    tensor_scalar_add for oc=1; bias is per-partition in this layout)
    DMA out [128, 2, MC] (1MB, Scalar HWDGE) -> out6[c]

fp32r: full-rate PE (1 cyc/row) at ~1.5e-4 rel RMS error (HW-measured;
fp32 is 4x slower, bf16 is 16x less accurate). DRAM inputs are declared
float32r with raw f32 bytes — HW rounds internally (verified equivalent
to explicit on-device rounding).
"""

import numpy as np

import concourse.bacc as bacc
import concourse.mybir as mybir
import concourse.tile as tile

F32 = mybir.dt.float32
F32R = mybir.dt.float32r
AF = mybir.ActivationFunctionType

# Problem geometry (hardcoded per spec).
P = 8          # paths == cores
BPP = 8        # batch per path
S = 4096
DIN = 256
DOUT = 256
M = BPP * S    # rows per core = 32768
MC = 1024      # m-columns per chunk (1MB DMA blocks)

_CACHE = {}


def build_nc(m=M, mc=MC):
    key = (m, mc)
    if key in _CACHE:
        return _CACHE[key]

    nchunk = m // mc
    assert nchunk * mc == m
    nh = mc // 512  # N=512 matmuls per (oc, ic)

    nc = bacc.Bacc("TRN2", target_bir_lowering=False, debug=False)
    xt_d = nc.dram_tensor("xt", [nchunk, 128, 2, mc], F32R, kind="ExternalInput")
    w_d = nc.dram_tensor("w", [2, 128, DOUT], F32R, kind="ExternalInput")
    bias_d = nc.dram_tensor("bias2", [2, 128], F32, kind="ExternalInput")
    out_d = nc.dram_tensor("out6", [nchunk, 128, 2, mc], F32, kind="ExternalOutput")

    with tile.TileContext(nc) as tc:
        with (
            tc.tile_pool(name="const", bufs=1) as const,
            tc.tile_pool(name="xin", bufs=3) as xin_pool,
            tc.tile_pool(name="oout", bufs=3) as oout_pool,
            tc.tile_pool(name="psum", bufs=2, space="PSUM") as psum,
        ):
            w_sb = const.tile([128, 2, DOUT], F32R, tag="w_sb")
            nc.sync.dma_start(w_sb[:], w_d[:].rearrange("c p n -> p c n"))
            bias_sb = const.tile([128, 2], F32, tag="bias_sb")
            nc.sync.dma_start(bias_sb[:], bias_d[:].rearrange("c p -> p c"))

            for c in range(nchunk):
                xt = xin_pool.tile([128, 2, mc], F32R, tag="xt")
                nc.sync.dma_start(xt[:], xt_d[c])
                osb = oout_pool.tile([128, 2, mc], F32, tag="osb")
                for oc in range(2):
                    pos = [
                        psum.tile(
                            [128, 512], F32, name=f"po{oc}{h}", tag=f"po{oc}{h}"
                        )
                        for h in range(nh)
                    ]
                    for ic in range(2):
                        for h in range(nh):
                            nc.tensor.matmul(
                                pos[h][:],
                                w_sb[:, ic, oc * 128 : (oc + 1) * 128],
                                xt[:, ic, h * 512 : (h + 1) * 512],
                                start=(ic == 0),
                                stop=(ic == 1),
                            )
                    for h in range(nh):
                        dst = osb[:, oc, h * 512 : (h + 1) * 512]
                        if oc == 0:
                            nc.scalar.activation(
                                dst, pos[h][:], AF.Identity,
                                bias=bias_sb[:, oc : oc + 1],
                            )
                        else:
                            nc.vector.tensor_scalar_add(
                                dst, pos[h][:], bias_sb[:, oc : oc + 1]
                            )
                nc.scalar.dma_start(out_d[c], osb[:])
    nc.compile()
    _CACHE[key] = nc
    return nc


def make_in_maps(x, weight_bank, bias, assigned_bits, m=M, mc=MC):
    """Host-side sharding + layout: per-core input dicts."""
    x = np.asarray(x, dtype=np.float32)
    weight_bank = np.asarray(weight_bank, dtype=np.float32)
    bias = np.asarray(bias, dtype=np.float32)
    idx = np.asarray(assigned_bits).astype(np.int64)

    nchunk = m // mc
    bias2 = np.ascontiguousarray(bias.reshape(2, 128))
    xs = x.reshape(P, m, DIN)
    in_maps = []
    for p in range(P):
        # xt[c, q, ic, j] = x_p[c*mc + j, ic*128 + q]
        xt = np.ascontiguousarray(
            xs[p].reshape(nchunk, mc, 2, 128).transpose(0, 3, 2, 1)
        )
        w_io = np.ascontiguousarray(weight_bank[idx[p]].T)  # [Din, Dout]
        in_maps.append(
            {
                "xt": xt,
                "w": w_io.reshape(2, 128, DOUT),
                "bias2": bias2,
            }
        )
    return in_maps


def assemble_out(results, m=M, mc=MC):
    nchunk = m // mc
    outs = []
    for r in results:
        # out6[c, q, oc, j] = out_p[c*mc + j, oc*128 + q]
        o6 = r["out6"].reshape(nchunk, 128, 2, mc)
        outs.append(o6.transpose(0, 3, 2, 1).reshape(m, DOUT))
    out = np.stack(outs)  # [P, m, DOUT]
    return np.ascontiguousarray(out.reshape(P * BPP, S, DOUT))


def kernel(x, weight_bank, bias, assigned_bits):
    from concourse.bass_utils import run_bass_kernel_spmd

    nc = build_nc()
    in_maps = make_in_maps(x, weight_bank, bias, assigned_bits)
    res = run_bass_kernel_spmd(nc, in_maps, core_ids=list(range(P)))
    return assemble_out(res.results)


# revision 6
# speedup vs baseline: 1.3404x; 1.0843x over previous
"""TRN2 Bass kernel for nn_MultiPrecisionLinear (moe_routing).

Reference computation:
    xs = x.reshape(P, bpp, S, Din)            # P=8 paths
    W  = weight_bank[assigned_bits]           # [P, Dout, Din]
    out = einsum('pbsi,poi->pbso', xs, W) + bias

Sharding: path-parallel. Core p holds path p's batch slice
[bpp*S, Din] = [32768, 256], its selected weight (as [Din, Dout]) and the
bias. All layout work happens on host so the device kernel is a pure
streaming matmul over fp32r:

  x is pre-transposed AND pre-chunked on host into contiguous 1MB blocks
  xt[c] = [128(i%128), 2(i//128), MC(m)]  -> each DMA reads one contiguous
  block, 8KB contiguous per partition (minimal descriptor count).

  per chunk c:
    DMA in  xt[c] (1MB, Sync HWDGE)
    8 fp32r matmuls (2 oc x 2 ic x 2 halves, N=512) -> out_T in PSUM
    bias add fused with PSUM->SBUF move (ACT Identity for oc=0, DVE
    tensor_scalar_add for oc=1; bias is per-partition in this layout)
    DMA out [128, 2, MC] (1MB, Scalar HWDGE) -> out6[c]

fp32r: full-rate PE (1 cyc/row) at ~1.5e-4 rel RMS error (HW-measured;
fp32 is 4x slower, bf16 is 16x less accurate). DRAM inputs are declared
float32r with raw f32 bytes — HW rounds internally (verified equivalent
to explicit on-device rounding).
"""

import numpy as np

import concourse.bacc as bacc
import concourse.mybir as mybir
import concourse.tile as tile

F32 = mybir.dt.float32
F32R = mybir.dt.float32r
AF = mybir.ActivationFunctionType

# Problem geometry (hardcoded per spec).
P = 8          # paths == cores
BPP = 8        # batch per path
S = 4096
DIN = 256
DOUT = 256
M = BPP * S    # rows per core = 32768
MC = 2048      # m-columns per chunk (2MB DMA blocks)

_CACHE = {}


def build_nc(m=M, mc=MC):
    key = (m, mc)
    if key in _CACHE:
        return _CACHE[key]

    nchunk = m // mc
    assert nchunk * mc == m
    nh = mc // 512  # N=512 matmuls per (oc, ic)

    nc = bacc.Bacc("TRN2", target_bir_lowering=False, debug=False)
    xt_d = nc.dram_tensor("xt", [nchunk, 128, 2, mc], F32R, kind="ExternalInput")
    w_d = nc.dram_tensor("w", [2, 128, DOUT], F32R, kind="ExternalInput")
    bias_d = nc.dram_tensor("bias2", [2, 128], F32, kind="ExternalInput")
    out_d = nc.dram_tensor("out6", [nchunk, 128, 2, mc], F32, kind="ExternalOutput")

    with tile.TileContext(nc) as tc:
        with (
            tc.tile_pool(name="const", bufs=1) as const,
            tc.tile_pool(name="xin", bufs=3) as xin_pool,
            tc.tile_pool(name="oout", bufs=3) as oout_pool,
            tc.tile_pool(name="psum", bufs=2, space="PSUM") as psum,
        ):
            # setup DMAs on SWDGE so the Sync HWDGE ring leads with chunk 0
            w_sb = const.tile([128, 2, DOUT], F32R, tag="w_sb")
            nc.gpsimd.dma_start(w_sb[:], w_d[:].rearrange("c p n -> p c n"))
            bias_sb = const.tile([128, 2], F32, tag="bias_sb")
            nc.gpsimd.dma_start(bias_sb[:], bias_d[:].rearrange("c p -> p c"))

            for c in range(nchunk):
                xt = xin_pool.tile([128, 2, mc], F32R, tag="xt")
                nc.sync.dma_start(xt[:], xt_d[c])
                osb = oout_pool.tile([128, 2, mc], F32, tag="osb")
                for oc in range(2):
                    for h in range(nh):
                        po = psum.tile(
                            [128, 512], F32, name=f"po{oc}{h}", tag=f"po{oc}{h % 2}"
                        )
                        for ic in range(2):
                            nc.tensor.matmul(
                                po[:],
                                w_sb[:, ic, oc * 128 : (oc + 1) * 128],
                                xt[:, ic, h * 512 : (h + 1) * 512],
                                start=(ic == 0),
                                stop=(ic == 1),
                            )
                        dst = osb[:, oc, h * 512 : (h + 1) * 512]
                        if oc == 0:
                            nc.scalar.activation(
                                dst, po[:], AF.Identity,
                                bias=bias_sb[:, oc : oc + 1],
                            )
                        else:
                            nc.vector.tensor_scalar_add(
                                dst, po[:], bias_sb[:, oc : oc + 1]
                            )
                nc.scalar.dma_start(out_d[c], osb[:])
    nc.compile()
    _CACHE[key] = nc
    return nc


def make_in_maps(x, weight_bank, bias, assigned_bits, m=M, mc=MC):
    """Host-side sharding + layout: per-core input dicts."""
    x = np.asarray(x, dtype=np.float32)
    weight_bank = np.asarray(weight_bank, dtype=np.float32)
    bias = np.asarray(bias, dtype=np.float32)
    idx = np.asarray(assigned_bits).astype(np.int64)

    nchunk = m // mc
    bias2 = np.ascontiguousarray(bias.reshape(2, 128))
    xs = x.reshape(P, m, DIN)
    in_maps = []
    for p in range(P):
        # xt[c, q, ic, j] = x_p[c*mc + j, ic*128 + q]
        xt = np.ascontiguousarray(
            xs[p].reshape(nchunk, mc, 2, 128).transpose(0, 3, 2, 1)
        )
        w_io = np.ascontiguousarray(weight_bank[idx[p]].T)  # [Din, Dout]
        in_maps.append(
            {
                "xt": xt,
                "w": w_io.reshape(2, 128, DOUT),
                "bias2": bias2,
            }
        )
    return in_maps


def assemble_out(results, m=M, mc=MC):
    nchunk = m // mc
    outs = []
    for r in results:
        # out6[c, q, oc, j] = out_p[c*mc + j, oc*128 + q]
        o6 = r["out6"].reshape(nchunk, 128, 2, mc)
        outs.append(o6.transpose(0, 3, 2, 1).reshape(m, DOUT))
    out = np.stack(outs)  # [P, m, DOUT]
    return np.ascontiguousarray(out.reshape(P * BPP, S, DOUT))


def kernel(x, weight_bank, bias, assigned_bits):
    from concourse.bass_utils import run_bass_kernel_spmd

    nc = build_nc()
    in_maps = make_in_maps(x, weight_bank, bias, assigned_bits)
    res = run_bass_kernel_spmd(nc, in_maps, core_ids=list(range(P)))
    return assemble_out(res.results)


# revision 8
# speedup vs baseline: 1.4162x; 1.0565x over previous
"""TRN2 Bass kernel for nn_MultiPrecisionLinear (moe_routing).

Reference computation:
    xs = x.reshape(P, bpp, S, Din)            # P=8 paths
    W  = weight_bank[assigned_bits]           # [P, Dout, Din]
    out = einsum('pbsi,poi->pbso', xs, W) + bias

Sharding: path-parallel. Core p holds path p's batch slice
[bpp*S, Din] = [32768, 256], its selected weight (as [Din, Dout]) and the
bias. All layout work happens on host so the device kernel is a pure
streaming matmul over fp32r:

  x is pre-transposed AND pre-chunked on host into contiguous 1MB blocks
  xt[c] = [128(i%128), 2(i//128), MC(m)]  -> each DMA reads one contiguous
  block, 8KB contiguous per partition (minimal descriptor count).

  per chunk c:
    DMA in  xt[c] (1MB, Sync HWDGE)
    8 fp32r matmuls (2 oc x 2 ic x 2 halves, N=512) -> out_T in PSUM
    bias add fused with PSUM->SBUF move (ACT Identity for oc=0, DVE
    tensor_scalar_add for oc=1; bias is per-partition in this layout)
    DMA out [128, 2, MC] (1MB, Scalar HWDGE) -> out6[c]

fp32r: full-rate PE (1 cyc/row) at ~1.5e-4 rel RMS error (HW-measured;
fp32 is 4x slower, bf16 is 16x less accurate). DRAM inputs are declared
float32r with raw f32 bytes — HW rounds internally (verified equivalent
to explicit on-device rounding).
"""

import numpy as np

import concourse.bacc as bacc
import concourse.mybir as mybir
import concourse.tile as tile

F32 = mybir.dt.float32
F32R = mybir.dt.float32r
AF = mybir.ActivationFunctionType

# Problem geometry (hardcoded per spec).
P = 8          # paths == cores
BPP = 8        # batch per path
S = 4096
DIN = 256
DOUT = 256
M = BPP * S    # rows per core = 32768
MC = 2048      # m-columns per chunk (2MB DMA blocks)

_CACHE = {}


def build_nc(m=M, mc=MC):
    key = (m, mc)
    if key in _CACHE:
        return _CACHE[key]

    nchunk = m // mc
    assert nchunk * mc == m
    nh = mc // 512  # N=512 matmuls per (oc, ic)

    nc = bacc.Bacc("TRN2", target_bir_lowering=False, debug=False)
    xt_d = nc.dram_tensor("xt", [nchunk, 128, 2, mc], F32R, kind="ExternalInput")
    w_d = nc.dram_tensor("w", [2, 128, DOUT], F32R, kind="ExternalInput")
    bias_d = nc.dram_tensor("bias2", [2, 128], F32, kind="ExternalInput")
    out_d = nc.dram_tensor("out6", [nchunk, 128, 2, mc], F32, kind="ExternalOutput")

    with tile.TileContext(nc) as tc:
        with (
            tc.tile_pool(name="const", bufs=1) as const,
            tc.tile_pool(name="xin", bufs=4) as xin_pool,
            tc.tile_pool(name="oout", bufs=3) as oout_pool,
            tc.tile_pool(name="psum", bufs=2, space="PSUM") as psum,
        ):
            # setup DMAs on the Scalar HWDGE ring (idle early; Sync leads
            # with chunk 0, and HWDGE beats SWDGE's slow Q7 spin-up)
            w_sb = const.tile([128, 2, DOUT], F32R, tag="w_sb")
            nc.scalar.dma_start(w_sb[:], w_d[:].rearrange("c p n -> p c n"))
            bias_sb = const.tile([128, 2], F32, tag="bias_sb")
            nc.scalar.dma_start(bias_sb[:], bias_d[:].rearrange("c p -> p c"))

            for c in range(nchunk):
                xt = xin_pool.tile([128, 2, mc], F32R, tag="xt")
                nc.sync.dma_start(xt[:], xt_d[c])
                osb = oout_pool.tile([128, 2, mc], F32, tag="osb")
                for oc in range(2):
                    for h in range(nh):
                        po = psum.tile(
                            [128, 512], F32, name=f"po{oc}{h}", tag=f"po{oc}{h % 2}"
                        )
                        for ic in range(2):
                            nc.tensor.matmul(
                                po[:],
                                w_sb[:, ic, oc * 128 : (oc + 1) * 128],
                                xt[:, ic, h * 512 : (h + 1) * 512],
                                start=(ic == 0),
                                stop=(ic == 1),
                            )
                        dst = osb[:, oc, h * 512 : (h + 1) * 512]
                        if oc == 0:
                            nc.scalar.activation(
                                dst, po[:], AF.Identity,
                                bias=bias_sb[:, oc : oc + 1],
                            )
                        else:
                            nc.vector.tensor_scalar_add(
                                dst, po[:], bias_sb[:, oc : oc + 1]
                            )
                nc.scalar.dma_start(out_d[c], osb[:])
    nc.compile()
    _CACHE[key] = nc
    return nc


def make_in_maps(x, weight_bank, bias, assigned_bits, m=M, mc=MC):
    """Host-side sharding + layout: per-core input dicts."""
    x = np.asarray(x, dtype=np.float32)
    weight_bank = np.asarray(weight_bank, dtype=np.float32)
    bias = np.asarray(bias, dtype=np.float32)
    idx = np.asarray(assigned_bits).astype(np.int64)

    nchunk = m // mc
    bias2 = np.ascontiguousarray(bias.reshape(2, 128))
    xs = x.reshape(P, m, DIN)
    in_maps = []
    for p in range(P):
        # xt[c, q, ic, j] = x_p[c*mc + j, ic*128 + q]
        xt = np.ascontiguousarray(
            xs[p].reshape(nchunk, mc, 2, 128).transpose(0, 3, 2, 1)
        )
        w_io = np.ascontiguousarray(weight_bank[idx[p]].T)  # [Din, Dout]
        in_maps.append(
            {
                "xt": xt,
                "w": w_io.reshape(2, 128, DOUT),
                "bias2": bias2,
            }
        )
    return in_maps


def assemble_out(results, m=M, mc=MC):
    nchunk = m // mc
    outs = []
    for r in results:
        # out6[c, q, oc, j] = out_p[c*mc + j, oc*128 + q]
        o6 = r["out6"].reshape(nchunk, 128, 2, mc)
        outs.append(o6.transpose(0, 3, 2, 1).reshape(m, DOUT))
    out = np.stack(outs)  # [P, m, DOUT]
    return np.ascontiguousarray(out.reshape(P * BPP, S, DOUT))


def kernel(x, weight_bank, bias, assigned_bits):
    from concourse.bass_utils import run_bass_kernel_spmd

    nc = build_nc()
    in_maps = make_in_maps(x, weight_bank, bias, assigned_bits)
    res = run_bass_kernel_spmd(nc, in_maps, core_ids=list(range(P)))
    return assemble_out(res.results)
